# revision 1
# baseline (speedup 1.0000x reference)
"""AttentionBlock (GroupNorm + single-head self-attention + proj + residual)
on 8 TRN2 NeuronCores. Data-parallel over batch: core i handles sample i.

Reference computation per sample (C=256, H=W=64, N=H*W=4096, G=32 groups):
  h    = groupnorm(x) * gamma + beta
  qkv  = w_qkv @ h + b_qkv              (1x1 conv == channel matmul)
  attn = softmax(q^T k / sqrt(C))       (N x N, never materialized in HBM)
  out  = x + w_proj @ (v @ attn^T) + b_proj

v3 structure (vs the 257us baseline):
  - w_proj folded into the v projection on the host: vp = (w_proj@w_v) h,
    so attn@v directly produces projected channels; the 32 proj matmuls and
    the bf16 att tiles disappear. out = x + (E vp)/den + bp_eff.
  - GroupNorm folded into the qkv weights on device (w' = w * sc per input
    channel), so x casts to fp8 on arrival and no h tensor is materialized.
  - bias algebra: k needs NO bias (a per-column score offset cancels in
    softmax); vp needs NO tensor bias (a constant vp offset rides through
    softmax into the output bias: bp_eff = bproj + wproj@bv + W_vp@sh, the
    last term via tiny on-device matmuls). Only q keeps a [P,1] ACT bias.
  - exp alternates ACT / custom-DVE *within* each m-pair:
    ((x*c0+1)^2*c1+c2)^8 ~= e^(x/16) on DVE (8-stage pipeline, 1.2% rel
    err), plain Exp on ACT. Neither engine paces the PE's 5-matmul pair.
  - q projections for blocks 2..7 are deferred into attention blocks 1..6,
    decompressing the block-0 qkv copy crunch.
  - x stays resident in SBUF for the residual (no 4MB re-read).
"""

import sys

for _p in ("/opt/trn_rl_repo", "/opt/pypackages"):
    if _p not in sys.path:
        sys.path.append(_p)

from contextlib import ExitStack

import numpy as np

import concourse.bass as bass
import concourse.tile as tile
from concourse import bacc, mybir
from concourse._compat import with_exitstack

B, C, H, W = 8, 256, 64, 64
N = H * W          # 4096
G = 32             # groups
GS = C // G        # 8 channels per group
EPS = 1e-5
P = 128
NCT = C // P       # 2 channel tiles
NBLK = 512         # attention n-block width
NB = N // NBLK     # 8
NM = N // P        # 32 m-tiles
SCALE = 1.0 / np.sqrt(np.float32(C))  # 1/16
WARMUP_MM = 56      # fp32 gmat matmuls to keep PE's HAM clock-gate warm

F32 = mybir.dt.float32
BF16 = mybir.dt.bfloat16
FP8 = mybir.dt.float8e4
DR = mybir.MatmulPerfMode.DoubleRow
AF = mybir.ActivationFunctionType
ALU = mybir.AluOpType

# ---- custom DVE op: out = ((x*c0 + 1)^2 * c1 + c2)^8 ~= exp(x * 8*c0) ----
# Exactly fills the v3 pipeline's 8 ALU stages; constants minimax-fitted so
# that with c0 = a*SCALE the op approximates exp(x*SCALE) to 1.2% rel err
# for |x*SCALE| <= 2.6 (scaled scores are ~N(0, 0.4^2); per-sample max |s|
# ~2.3). The fp8 output rounding (~3%) dominates this.
_EXP_A = 0.12251085
_EXP_C1 = 0.51681271
_EXP_C2 = 0.4835532
EXP_C0 = float(_EXP_A * SCALE)


def _exp8_ref(in0, in1, s0, s1, imm2):
    u = in0.astype(np.float32) * s0 + 1.0
    w = u * u * s1 + imm2
    t = w * w
    t = t * t
    return t * t


def _register_exp8():
    import concourse.dve_ops as dve_ops
    from concourse.dve_ops import DveOp
    from concourse.dve_spec import C0, C1, C2, One, Spec, Src0
    from concourse.dve_spec import lower as dve_lower
    from concourse.dve_uop import DveOpSpec

    if any(op.name == "EXP8_ANT" for op in dve_ops.OPS):
        return next(op for op in dve_ops.OPS if op.name == "EXP8_ANT")
    body = Src0 * C0 + One
    body = body * body
    body = body * C1 + C2
    body = body * body
    body = body * body
    body = body * body
    spec = Spec(body=body, reference=_exp8_ref)
    row = max(dve_ops._SUB_OPCODE_FOR_NAME.values()) + 1
    assert row < 0x20
    sha = {
        ver: DveOpSpec(
            name="EXP8_ANT", opcode=row, uops=dve_lower(spec, ver=ver),
            rd1_en=False,
        ).sha(ver)
        for ver in ("v3",)
    }
    op = DveOp("EXP8_ANT", spec, subdim=False, uops_sha=sha)
    dve_ops.OPS.append(op)
    dve_ops.CUSTOM_DVE_SPECS[op.name] = spec
    dve_ops._SUB_OPCODE_FOR_NAME[op.name] = row
    return op


EXP8 = _register_exp8()


def _group_mat() -> np.ndarray:
    """A[c, c'] = 1/GS if c and c' are in the same group (within a 128-chan
    tile); A^T @ t group-averages per-channel stats in one PE matmul."""
    a = np.zeros((P, P), np.float32)
    for g in range(P // GS):
        a[g * GS:(g + 1) * GS, g * GS:(g + 1) * GS] = 1.0 / GS
    return a


def _col(ap_1d, lo, hi):
    """Slice a 1-D DRAM AP into a [hi-lo, 1] AP (partition dim x 1)."""
    sl = ap_1d[lo:hi]
    return bass.AP(tensor=sl.tensor, offset=sl.offset, ap=[*sl.ap, [1, 1]])


@with_exitstack
def emit_kernel(ctx: ExitStack, tc: tile.TileContext, out_d, x_d, wqkvT_d,
                bqkv_d, bproj_d, gamma_d, beta_d, gmat_d):
    nc = tc.nc

    big = ctx.enter_context(tc.tile_pool(name="big", bufs=1))
    small = ctx.enter_context(tc.tile_pool(name="small", bufs=1))
    work = ctx.enter_context(tc.tile_pool(name="work", bufs=3))
    work2 = ctx.enter_context(tc.tile_pool(name="work2", bufs=3))
    tdiv = ctx.enter_context(tc.tile_pool(name="tdiv", bufs=4))
    stage = ctx.enter_context(tc.tile_pool(name="stage", bufs=4))
    ps_s = ctx.enter_context(tc.tile_pool(name="ps_s", bufs=3, space="PSUM"))
    ps_av0 = ctx.enter_context(tc.tile_pool(name="ps_av0", bufs=2, space="PSUM"))
    ps_av1 = ctx.enter_context(tc.tile_pool(name="ps_av1", bufs=2, space="PSUM"))
    ps_sum = ctx.enter_context(tc.tile_pool(name="ps_sum", bufs=1, space="PSUM"))

    # ---- gmat first: its DVE copy feeds PE warmup matmuls that keep the
    # HAM clock-gate warm while x loads / groupnorm stats run ----
    gmat_f = small.tile([P, P], F32, tag="gmatf")
    nc.sync.dma_start(gmat_f, gmat_d[:, :])
    gmat_sb = small.tile([P, P], F32, tag="gmat")
    nc.vector.tensor_copy(gmat_sb, gmat_f)
    for w in range(WARMUP_MM):
        pw = ps_s.tile([P, P], F32, tag="s", name=f"warm{w}")
        nc.tensor.matmul(pw, lhsT=gmat_sb, rhs=gmat_sb, start=True, stop=True)

    # ---- constants / weights to SBUF ----
    eps_t = small.tile([P, 1], F32, tag="eps")
    nc.vector.memset(eps_t, float(EPS))
    # preload the Sqrt act table while the engines boot (Sqrt and Exp live
    # in different table sets; each implicit load costs 1.28us on ACT)
    sqrt_dummy = small.tile([P, 1], F32, tag="sqrt_dummy")
    nc.scalar.activation(sqrt_dummy, eps_t, AF.Sqrt, bias=eps_t)
    gamma_t = []
    beta_t = []
    bp_t = []
    for ct in range(NCT):
        gt = small.tile([P, 1], F32, tag=f"gamma{ct}")
        nc.sync.dma_start(gt, _col(gamma_d, ct * P, (ct + 1) * P))
        gamma_t.append(gt)
        bt = small.tile([P, 1], F32, tag=f"beta{ct}")
        nc.sync.dma_start(bt, _col(beta_d, ct * P, (ct + 1) * P))
        beta_t.append(bt)
        t = small.tile([P, 1], F32, tag=f"bp{ct}")
        nc.sync.dma_start(t, _col(bproj_d, ct * P, (ct + 1) * P))
        bp_t.append(t)
    bq_in = []
    for o in range(NCT):  # only q's bias survives the softmax algebra
        t = small.tile([P, 1], F32, tag=f"bq{o}")
        nc.sync.dma_start(t, _col(bqkv_d, o * P, (o + 1) * P))
        bq_in.append(t)

    # ---- load x (3-way split across the SP / GPSIMD / ACT DGE queues);
    # bn_stats (DVE) + fp8 cast (ACT) interleave with chunk arrival. The
    # scalar queue takes the LAST 5 chunk indices so stats emission order
    # matches arrival order, and wqf (0.79MB, needed only at fold time
    # ~30us) loads after them. ----
    NXC = 4            # x load chunks per channel tile (1024 cols each:
    XCW = N // NXC     # 4KB DMA descriptors, half the descriptor count)
    x_sb = []
    stats_t = []
    for ct in range(NCT):
        xt = big.tile([P, N], F32, tag=f"x{ct}", name=f"x{ct}")
        x_sb.append(xt)
        stats_t.append(small.tile([P, NB, 6], F32, tag=f"bnst{ct}",
                                  name=f"bnst{ct}"))
    x8 = big.tile([P, 2, N], FP8, tag="x8")
    qs_order = [nc.sync, nc.gpsimd, nc.sync, nc.gpsimd, nc.sync, nc.gpsimd,
                nc.scalar, nc.scalar]
    for j in range(NXC):
        for ct in range(NCT):
            eng = qs_order[2 * j + ct]
            eng.dma_start(x_sb[ct][:, j * XCW:(j + 1) * XCW],
                          x_d[ct * P:(ct + 1) * P, j * XCW:(j + 1) * XCW])
    wqf = small.tile([P, 2, 3 * C], F32, tag="wqkvTf", name="wqf")
    nc.scalar.dma_start(wqf, wqkvT_d[:, :, :])
    for j in range(NB):
        for ct in range(NCT):
            csl = slice(j * NBLK, (j + 1) * NBLK)
            nc.vector.bn_stats(stats_t[ct][:, j, :], x_sb[ct][:, csl])
            nc.scalar.copy(x8[:, ct, csl], x_sb[ct][:, csl])

    # ---- GN stats -> per-channel scale/shift (h = x*sc + sh) ----
    scale_sh = []
    for ct in range(NCT):
        mv = small.tile([P, 2], F32, tag=f"mv{ct}")
        nc.vector.bn_aggr(mv, stats_t[ct])
        # t = [mean_c, E[x^2]_c]
        t = small.tile([P, 2], F32, tag=f"t{ct}")
        nc.vector.tensor_copy(t[:, 0:1], mv[:, 0:1])
        nc.vector.tensor_mul(t[:, 1:2], mv[:, 0:1], mv[:, 0:1])
        nc.vector.tensor_add(t[:, 1:2], t[:, 1:2], mv[:, 1:2])
        # group-average + broadcast back to channels via PE
        psg = ps_s.tile([P, 2], F32, tag="s")
        nc.tensor.matmul(psg, lhsT=gmat_sb, rhs=t, start=True, stop=True)
        g_sb = small.tile([P, 2], F32, tag=f"g{ct}")
        nc.vector.tensor_copy(g_sb, psg)
        # scale = gamma * rsqrt(var + eps);  shift = beta - group_mean * scale
        tmp = small.tile([P, 1], F32, tag=f"tmp{ct}")
        sc = small.tile([P, 1], F32, tag=f"sc{ct}")
        sh = small.tile([P, 1], F32, tag=f"sh{ct}")
        nc.vector.tensor_mul(tmp, g_sb[:, 0:1], g_sb[:, 0:1])
        nc.vector.tensor_tensor(tmp, g_sb[:, 1:2], tmp, ALU.subtract)  # var
        nc.scalar.activation(tmp, tmp, AF.Sqrt, bias=eps_t)
        nc.vector.reciprocal(tmp, tmp)                                 # rstd
        nc.vector.tensor_mul(sc, tmp, gamma_t[ct])
        nc.vector.tensor_mul(tmp, g_sb[:, 0:1], sc)
        nc.vector.tensor_tensor(sh, beta_t[ct], tmp, ALU.subtract)
        scale_sh.append((sc, sh))
    # switch the ACT table back to Exp now, off the critical path, so the
    # first softmax exp doesn't pay the 1.28us implicit load
    nc.scalar.activation(sqrt_dummy, eps_t, AF.Exp, scale=1.0)

    # ---- fold GN into the weights: wq8[c,:] = wqf[c,:] * sc[c] in fp8;
    # q bias' = W_q @ sh + b_q and bp_eff += W_vp @ sh via tiny matmuls ----
    wq8 = small.tile([P, 2, 3 * C], FP8, tag="wqkvT8", name="wq8")
    for ct in range(NCT):
        nc.vector.tensor_scalar(wq8[:, ct, :], wqf[:, ct, :], scale_sh[ct][0],
                                None, op0=ALU.mult)
    bias_q = []
    for o in range(NCT):
        psb = ps_s.tile([P, NBLK], F32, tag="s")
        for ct in range(NCT):
            nc.tensor.matmul(psb[:, 0:1], lhsT=wqf[:, ct, o * P:(o + 1) * P],
                             rhs=scale_sh[ct][1], start=(ct == 0),
                             stop=(ct == NCT - 1))
        bj = small.tile([P, 1], F32, tag=f"biasq{o}")
        nc.vector.tensor_add(bj, psb[:, 0:1], bq_in[o])
        bias_q.append(bj)
    bp_eff = []
    for o in range(NCT):
        psb = ps_s.tile([P, NBLK], F32, tag="s")
        for ct in range(NCT):
            nc.tensor.matmul(
                psb[:, 0:1],
                lhsT=wqf[:, ct, 2 * C + o * P:2 * C + (o + 1) * P],
                rhs=scale_sh[ct][1], start=(ct == 0), stop=(ct == NCT - 1))
        bj = small.tile([P, 1], F32, tag=f"bpe{o}")
        nc.vector.tensor_add(bj, psb[:, 0:1], bp_t[o])
        bp_eff.append(bj)

    # ---- qkv projections (GN pre-folded, so rhs is x8 directly). q/k land
    # in fp8 [128, 2, N] (channel-half on the middle dim) and vp in fp8
    # m-pair-interleaved [128, 2, 272] tiles so the attention matmuls can use
    # fp8 DoubleRow (2 values/PE-cell -> one matmul contracts 256). vp is the
    # w_proj-fused v projection; col 256 = ones (softmax denominators). ----
    q2 = big.tile([P, 2, N], FP8, tag="q2")
    k2 = big.tile([P, 2, N], FP8, tag="k2")
    VTW = 272
    vt_lo = big.tile([P, NM // 4, 2, VTW], FP8, tag="vt0", name="vt_lo")
    vt_hi = big.tile([P, NM // 4, 2, VTW], FP8, tag="vt1", name="vt_hi")
    # pre-fill the ones columns once (strided memsets, off-critical-path)
    nc.gpsimd.memset(vt_lo[:, :, :, C:C + 1], 1.0)
    nc.gpsimd.memset(vt_hi[:, :, :, C:C + 1], 1.0)

    def vt2(pair):
        return (vt_lo[:, pair] if pair < NM // 4
                else vt_hi[:, pair - NM // 4])

    def emit_q_blk(blk, only_o=None):
        """q projection for block blk: 2 matmuls + 2 biased ACT copies.
        Deferrable (per channel-half) to just before block blk needs q2."""
        bsl = slice(blk * NBLK, (blk + 1) * NBLK)
        for o in range(NCT):
            if only_o is not None and o != only_o:
                continue
            ps = ps_s.tile([P, NBLK], F32, tag="s")
            nc.tensor.matmul(
                ps, lhsT=wq8[:, :, o * P:(o + 1) * P], rhs=x8[:, :, bsl],
                start=True, stop=True, perf_mode=DR)
            nc.scalar.activation(q2[:, o, bsl], ps, AF.Identity,
                                 bias=bias_q[o], scale=1.0)

    def emit_kv_blk(blk):
        """k + vp projections for block blk (biases cancel / fold away, so
        all copies are plain psum->fp8 casts, split across ACT and DVE)."""
        bsl = slice(blk * NBLK, (blk + 1) * NBLK)
        for o in range(NCT):
            ps = ps_s.tile([P, NBLK], F32, tag="s")
            nc.tensor.matmul(
                ps, lhsT=wq8[:, :, C + o * P:C + (o + 1) * P],
                rhs=x8[:, :, bsl], start=True, stop=True, perf_mode=DR)
            if o == 0:
                nc.scalar.copy(k2[:, o, bsl], ps)
            else:
                nc.vector.tensor_copy(k2[:, o, bsl], ps)
        for m in range(4 * blk, 4 * blk + 4):
            ps = ps_s.tile([P, NBLK], F32, tag="s")
            nc.tensor.matmul(
                ps[:, 0:C], lhsT=x8[:, :, m * P:(m + 1) * P],
                rhs=wq8[:, :, 2 * C:3 * C],
                start=True, stop=True, perf_mode=DR)
            dst = vt2(m // 2)[:, m % 2]
            if m % 2 == 0:
                nc.scalar.copy(dst[:, 0:C], ps[:, 0:C])
            else:
                nc.vector.tensor_copy(dst[:, 0:C], ps[:, 0:C])

    # ---- softmax divide + output helpers ----
    def emit_div_a(pend):
        pav0, pav1, psum, nb = pend
        sums_sb = work2.tile([1, NBLK], F32, tag="sums")
        nc.scalar.activation(sums_sb, psum, AF.Copy, bias=0.0)
        bc2 = work2.tile([P, NBLK], F32, tag="bc2")
        nc.gpsimd.partition_broadcast(bc2, sums_sb)
        bc_sb = work2.tile([P, NBLK], F32, tag="bc")
        nc.vector.reciprocal_approx_fast(bc_sb, bc2)
        return bc_sb

    def emit_div_b(pend, bc_sb, o):
        """One output channel-half: divide + bias + residual + store.
        Split into two calls so the DVE burst spreads across the block."""
        pav = pend[o]
        nb = pend[3]
        nsl = slice(nb * NBLK, (nb + 1) * NBLK)
        t = tdiv.tile([P, NBLK], F32, tag="t")
        nc.vector.tensor_mul(t, pav, bc_sb)
        st = stage.tile([P, NBLK], F32, tag="st")
        nc.vector.scalar_tensor_tensor(st, t, bp_eff[o], x_sb[o][:, nsl],
                                       op0=ALU.add, op1=ALU.add)
        eng = nc.sync if o == 0 else nc.gpsimd
        eng.dma_start(out_d[o * P:(o + 1) * P, nsl], st)

    # ---- global software-pipelined attention loop over gp = nb*16 + pair.
    # At iteration gp we emit: exps(gp+1) (engine queues run them while PE
    # works), the score matmuls for gp+2 (split around the avs so the bank
    # freed by exp0(gp+1) is reused late), and the av matmuls for gp (whose
    # e2 was exp'd during iteration gp-1 -> a full pair-period of exp slack,
    # so exp latency never stalls the PE). ----
    NPAIR = NM // 2          # 16 pairs per block
    NGP = NB * NPAIR         # 128
    ps_m = {}
    e2_pend = {}
    blk_tiles = {}

    def emit_scores(gp, half):
        if gp >= NGP:
            return
        nb, p = divmod(gp, NPAIR)
        m = 2 * p + half
        ps = ps_s.tile([P, NBLK], F32, tag="s")
        nc.tensor.matmul(ps, lhsT=k2[:, :, m * P:(m + 1) * P],
                         rhs=q2[:, :, nb * NBLK:(nb + 1) * NBLK],
                         start=True, stop=True, perf_mode=DR)
        ps_m[(gp, half)] = ps

    def dve_pair(gp):
        # Runs of 2 pairs per engine: short enough that the queue lag stays
        # within the 3-bank slack, long enough that ACT and DVE are not hot
        # simultaneously (concurrent ACT+DVE activity drops the chip clock
        # from 2.4 to 2.0 GHz -- measured 379ns vs 454ns matmuls).
        nb, p = divmod(gp, NPAIR)
        if nb == 0:
            return p % 4 == 1
        return p % 4 >= 2

    def emit_exps(gp):
        if gp >= NGP:
            return
        on_dve = dve_pair(gp)
        e2 = work.tile([P, 2, NBLK], FP8, tag="e")
        for half in range(2):
            ps = ps_m.pop((gp, half))
            if on_dve:
                nc.vector._custom_dve(EXP8, out=e2[:, half], in0=ps,
                                      s0=EXP_C0, s1=_EXP_C1, imm2=_EXP_C2)
            else:
                nc.scalar.activation(e2[:, half], ps, AF.Exp,
                                     scale=float(SCALE))
        e2_pend[gp] = e2

    def emit_avs(gp):
        nb, p = divmod(gp, NPAIR)
        e2 = e2_pend.pop(gp)
        pav0, pav1, psum = blk_tiles[nb]
        first, last = (p == 0), (p == NPAIR - 1)
        vtp = vt2(p)
        nc.tensor.matmul(pav0, lhsT=vtp[:, :, 0:P], rhs=e2,
                         start=first, stop=last, perf_mode=DR)
        nc.tensor.matmul(pav1, lhsT=vtp[:, :, P:2 * P], rhs=e2,
                         start=first, stop=last, perf_mode=DR)
        nc.tensor.matmul(psum, lhsT=vtp[:, :, 2 * P:2 * P + 1], rhs=e2,
                         start=first, stop=last, perf_mode=DR)

    def new_blk_tiles(nb):
        pav0 = ps_av0.tile([P, NBLK], F32, tag="av0", name=f"av0_{nb}")
        pav1 = ps_av1.tile([P, NBLK], F32, tag="av1", name=f"av1_{nb}")
        psum = ps_sum.tile([1, NBLK], F32, tag="sum", name=f"sum_{nb}")
        blk_tiles[nb] = (pav0, pav1, psum)

    # Fused phase gating: kv block b unlocks k2 m-tiles < 4(b+1) and vt
    # pairs < 2(b+1); at iteration gp of block 0 the scores reach m-tile
    # 2*gp+5 and the avs read vt pair gp, both covered once kv blocks
    # <= gp/2 + 2 are in.
    emit_q_blk(0)
    emit_kv_blk(0)
    emit_kv_blk(1)
    emit_scores(0, 0)
    emit_scores(0, 1)
    emit_exps(0)
    emit_scores(1, 0)
    emit_scores(1, 1)
    emit_q_blk(1)

    state = {"pend": None, "bc_prev": None}
    for gp in range(NGP):
        nb, p = divmod(gp, NPAIR)
        if nb not in blk_tiles:
            new_blk_tiles(nb)
        if nb == 0:
            if p % 2 == 0 and 2 + p // 2 < NB:
                emit_kv_blk(2 + p // 2)
            if p == 12:
                emit_q_blk(2)
        emit_exps(gp + 1)
        emit_scores(gp + 2, 0)
        # div_b (DVE work) lands on ACT-exp pairs; deferred q copies (ACT
        # work) land on DVE-exp pairs.
        if p == 4 and nb > 0:
            emit_div_b(state["pend"], state["bc_prev"], 0)
        if p == 8 and nb > 0:
            emit_div_b(state["pend"], state["bc_prev"], 1)
        if p == 2 and nb >= 2 and nb + 1 < NB:
            emit_q_blk(nb + 1, only_o=0)
        if p == 6 and nb >= 2 and nb + 1 < NB:
            emit_q_blk(nb + 1, only_o=1)
        emit_avs(gp)
        emit_scores(gp + 2, 1)
        if p == NPAIR - 1:
            # div_a emitted before the next block's first den matmul can
            # touch the single-buffered ps_sum bank
            state["pend"] = (*blk_tiles.pop(nb), nb)
            state["bc_prev"] = emit_div_a(state["pend"])
    emit_div_b(state["pend"], state["bc_prev"], 0)
    emit_div_b(state["pend"], state["bc_prev"], 1)


def build_nc() -> bass.Bass:
    nc = bacc.Bacc("TRN2", target_bir_lowering=False, debug=False)
    x = nc.dram_tensor("x", [C, N], F32, kind="ExternalInput")
    wqkvT = nc.dram_tensor("wqkvT", [P, 2, 3 * C], F32, kind="ExternalInput")
    bqkv = nc.dram_tensor("bqkv", [3 * C], F32, kind="ExternalInput")
    bproj = nc.dram_tensor("bproj", [C], F32, kind="ExternalInput")
    gamma = nc.dram_tensor("gamma", [C], F32, kind="ExternalInput")
    beta = nc.dram_tensor("beta", [C], F32, kind="ExternalInput")
    gmat = nc.dram_tensor("gmat", [P, P], F32, kind="ExternalInput")
    out = nc.dram_tensor("out", [C, N], F32, kind="ExternalOutput")
    with tile.TileContext(nc) as tc:
        emit_kernel(tc, out.ap(), x.ap(), wqkvT.ap(), bqkv.ap(),
                    bproj.ap(), gamma.ap(), beta.ap(), gmat.ap())
    nc.compile()
    return nc


_NC_CACHE: list = []


def _in_maps(x, gamma, beta, w_qkv, b_qkv, w_proj, b_proj):
    f = lambda a: np.ascontiguousarray(np.asarray(a, dtype=np.float32))
    xs = f(x).reshape(B, C, N)
    w_qkv = np.asarray(w_qkv, dtype=np.float64)
    w_proj = np.asarray(w_proj, dtype=np.float64)
    b_qkv = np.asarray(b_qkv, dtype=np.float64)
    b_proj = np.asarray(b_proj, dtype=np.float64)
    # fuse w_proj into the v projection; its bias rides into bproj (softmax
    # rows sum to 1, so a constant vp offset is a constant output offset)
    w_fused = np.concatenate(
        [w_qkv[0:2 * C], w_proj @ w_qkv[2 * C:3 * C]], axis=0)
    bp_eff = b_proj + w_proj @ b_qkv[2 * C:3 * C]
    base = {
        "wqkvT": f(w_fused.T.reshape(2, P, 3 * C).transpose(1, 0, 2)),
        "bqkv": f(b_qkv),
        "bproj": f(bp_eff),
        "gamma": f(gamma),
        "beta": f(beta),
        "gmat": _group_mat(),
    }
    return [{**base, "x": np.ascontiguousarray(xs[i])} for i in range(B)]


def run_spmd(x, gamma, beta, w_qkv, b_qkv, w_proj, b_proj, **kwargs):
    from concourse.bass_utils import run_bass_kernel_spmd

    if not _NC_CACHE:
        _NC_CACHE.append(build_nc())
    nc = _NC_CACHE[0]
    maps = _in_maps(x, gamma, beta, w_qkv, b_qkv, w_proj, b_proj)
    res = run_bass_kernel_spmd(nc, maps, core_ids=list(range(B)), **kwargs)
    out = np.stack([res.results[i]["out"] for i in range(B)])
    return out.reshape(B, C, H, W), res


def kernel(x, gamma, beta, w_qkv, b_qkv, w_proj, b_proj) -> np.ndarray:
    out, _ = run_spmd(x, gamma, beta, w_qkv, b_qkv, w_proj, b_proj)
    return out



# revision 14
# speedup vs baseline: 1.0177x; 1.0177x over previous
"""AttentionBlock (GroupNorm + single-head self-attention + proj + residual)
on 8 TRN2 NeuronCores. Data-parallel over batch: core i handles sample i.

Reference computation per sample (C=256, H=W=64, N=H*W=4096, G=32 groups):
  h    = groupnorm(x) * gamma + beta
  qkv  = w_qkv @ h + b_qkv              (1x1 conv == channel matmul)
  attn = softmax(q^T k / sqrt(C))       (N x N, never materialized in HBM)
  out  = x + w_proj @ (v @ attn^T) + b_proj

v3 structure (vs the 257us baseline):
  - w_proj folded into the v projection on the host: vp = (w_proj@w_v) h,
    so attn@v directly produces projected channels; the 32 proj matmuls and
    the bf16 att tiles disappear. out = x + (E vp)/den + bp_eff.
  - GroupNorm folded into the qkv weights on device (w' = w * sc per input
    channel), so x casts to fp8 on arrival and no h tensor is materialized.
  - bias algebra: k needs NO bias (a per-column score offset cancels in
    softmax); vp needs NO tensor bias (a constant vp offset rides through
    softmax into the output bias: bp_eff = bproj + wproj@bv + W_vp@sh, the
    last term via tiny on-device matmuls). Only q keeps a [P,1] ACT bias.
  - exp alternates ACT / custom-DVE *within* each m-pair:
    ((x*c0+1)^2*c1+c2)^8 ~= e^(x/16) on DVE (8-stage pipeline, 1.2% rel
    err), plain Exp on ACT. Neither engine paces the PE's 5-matmul pair.
  - q projections for blocks 2..7 are deferred into attention blocks 1..6,
    decompressing the block-0 qkv copy crunch.
  - x stays resident in SBUF for the residual (no 4MB re-read).
"""

import sys

for _p in ("/opt/trn_rl_repo", "/opt/pypackages"):
    if _p not in sys.path:
        sys.path.append(_p)

from contextlib import ExitStack

import numpy as np

import concourse.bass as bass
import concourse.tile as tile
from concourse import bacc, mybir
from concourse._compat import with_exitstack

B, C, H, W = 8, 256, 64, 64
N = H * W          # 4096
G = 32             # groups
GS = C // G        # 8 channels per group
EPS = 1e-5
P = 128
NCT = C // P       # 2 channel tiles
NBLK = 512         # attention n-block width
NB = N // NBLK     # 8
NM = N // P        # 32 m-tiles
SCALE = 1.0 / np.sqrt(np.float32(C))  # 1/16
WARMUP_MM = 44      # fp32 gmat matmuls to keep PE's HAM clock-gate warm

F32 = mybir.dt.float32
BF16 = mybir.dt.bfloat16
FP8 = mybir.dt.float8e4
DR = mybir.MatmulPerfMode.DoubleRow
AF = mybir.ActivationFunctionType
ALU = mybir.AluOpType

# ---- custom DVE op: out = ((x*c0 + 1)^2 * c1 + c2)^8 ~= exp(x * 8*c0) ----
# Exactly fills the v3 pipeline's 8 ALU stages; constants minimax-fitted so
# that with c0 = a*SCALE the op approximates exp(x*SCALE) to 1.2% rel err
# for |x*SCALE| <= 2.6 (scaled scores are ~N(0, 0.4^2); per-sample max |s|
# ~2.3). The fp8 output rounding (~3%) dominates this.
_EXP_A = 0.12251085
_EXP_C1 = 0.51681271
_EXP_C2 = 0.4835532
EXP_C0 = float(_EXP_A * SCALE)


def _exp8_ref(in0, in1, s0, s1, imm2):
    u = in0.astype(np.float32) * s0 + 1.0
    w = u * u * s1 + imm2
    t = w * w
    t = t * t
    return t * t


def _register_exp8():
    import concourse.dve_ops as dve_ops
    from concourse.dve_ops import DveOp
    from concourse.dve_spec import C0, C1, C2, One, Spec, Src0
    from concourse.dve_spec import lower as dve_lower
    from concourse.dve_uop import DveOpSpec

    if any(op.name == "EXP8_ANT" for op in dve_ops.OPS):
        return next(op for op in dve_ops.OPS if op.name == "EXP8_ANT")
    body = Src0 * C0 + One
    body = body * body
    body = body * C1 + C2
    body = body * body
    body = body * body
    body = body * body
    spec = Spec(body=body, reference=_exp8_ref)
    row = max(dve_ops._SUB_OPCODE_FOR_NAME.values()) + 1
    assert row < 0x20
    sha = {
        ver: DveOpSpec(
            name="EXP8_ANT", opcode=row, uops=dve_lower(spec, ver=ver),
            rd1_en=False,
        ).sha(ver)
        for ver in ("v3",)
    }
    op = DveOp("EXP8_ANT", spec, subdim=False, uops_sha=sha)
    dve_ops.OPS.append(op)
    dve_ops.CUSTOM_DVE_SPECS[op.name] = spec
    dve_ops._SUB_OPCODE_FOR_NAME[op.name] = row
    return op


EXP8 = _register_exp8()


def _group_mat() -> np.ndarray:
    """A[c, c'] = 1/GS if c and c' are in the same group (within a 128-chan
    tile); A^T @ t group-averages per-channel stats in one PE matmul."""
    a = np.zeros((P, P), np.float32)
    for g in range(P // GS):
        a[g * GS:(g + 1) * GS, g * GS:(g + 1) * GS] = 1.0 / GS
    return a


def _col(ap_1d, lo, hi):
    """Slice a 1-D DRAM AP into a [hi-lo, 1] AP (partition dim x 1)."""
    sl = ap_1d[lo:hi]
    return bass.AP(tensor=sl.tensor, offset=sl.offset, ap=[*sl.ap, [1, 1]])


@with_exitstack
def emit_kernel(ctx: ExitStack, tc: tile.TileContext, out_d, x_d, wqkvT_d,
                consts_d, gmat_d):
    nc = tc.nc

    big = ctx.enter_context(tc.tile_pool(name="big", bufs=1))
    small = ctx.enter_context(tc.tile_pool(name="small", bufs=1))
    work = ctx.enter_context(tc.tile_pool(name="work", bufs=3))
    work2 = ctx.enter_context(tc.tile_pool(name="work2", bufs=3))
    tdiv = ctx.enter_context(tc.tile_pool(name="tdiv", bufs=4))
    stage = ctx.enter_context(tc.tile_pool(name="stage", bufs=4))
    ps_s = ctx.enter_context(tc.tile_pool(name="ps_s", bufs=3, space="PSUM"))
    ps_av0 = ctx.enter_context(tc.tile_pool(name="ps_av0", bufs=2, space="PSUM"))
    ps_av1 = ctx.enter_context(tc.tile_pool(name="ps_av1", bufs=2, space="PSUM"))
    ps_sum = ctx.enter_context(tc.tile_pool(name="ps_sum", bufs=1, space="PSUM"))

    # ---- scalar queue first: gmat (64KB, feeds PE warmups at ~9us), the
    # packed consts [128, 8] (gamma|beta|bp|bq -- eight separate [128,1]
    # column DMAs cost ~1.1us of issue each), then wqf. All issued before
    # the sqrt preload so its ACT table load can't delay them. ----
    gmat_f = small.tile([P, P], F32, tag="gmatf")
    nc.scalar.dma_start(gmat_f, gmat_d[:, :])
    consts_t = small.tile([P, 8], F32, tag="consts")
    nc.scalar.dma_start(consts_t, consts_d[:, :])
    wqf = small.tile([P, 2, 3 * C], F32, tag="wqkvTf", name="wqf")
    nc.scalar.dma_start(wqf, wqkvT_d[:, :, :])
    gamma_t = [consts_t[:, ct:ct + 1] for ct in range(NCT)]
    beta_t = [consts_t[:, 2 + ct:3 + ct] for ct in range(NCT)]
    bp_t = [consts_t[:, 4 + ct:5 + ct] for ct in range(NCT)]
    bq_in = [consts_t[:, 6 + o:7 + o] for o in range(NCT)]

    gmat_sb = small.tile([P, P], F32, tag="gmat")
    nc.vector.tensor_copy(gmat_sb, gmat_f)

    eps_t = small.tile([P, 1], F32, tag="eps")
    nc.vector.memset(eps_t, float(EPS))
    # preload the Sqrt act table while the engines boot (Sqrt and Exp live
    # in different table sets; each implicit load costs 1.28us on ACT)
    sqrt_dummy = small.tile([P, 1], F32, tag="sqrt_dummy")
    nc.scalar.activation(sqrt_dummy, eps_t, AF.Sqrt, bias=eps_t)

    # ---- load x in 2048-col chunks (8KB per-partition descriptors): the
    # 3 DMA rings ran at only ~79GB/s each with 4KB descriptors, so bigger
    # descriptors are the only lever on the ~20us load. sync + gpsimd take
    # 2MB of x each; scalar carries gmat/consts/wqf (0.85MB). bn_stats
    # (DVE) + fp8 cast (ACT) chase chunk arrival. ----
    NXC = 2            # x load chunks per channel tile
    XCW = N // NXC
    x_sb = []
    stats_t = []
    for ct in range(NCT):
        xt = big.tile([P, N], F32, tag=f"x{ct}", name=f"x{ct}")
        x_sb.append(xt)
        stats_t.append(small.tile([P, NB, 6], F32, tag=f"bnst{ct}",
                                  name=f"bnst{ct}"))
    x8 = big.tile([P, 2, N], FP8, tag="x8")
    for j in range(NXC):
        for ct in range(NCT):
            eng = nc.sync if ct == 0 else nc.gpsimd
            eng.dma_start(x_sb[ct][:, j * XCW:(j + 1) * XCW],
                          x_d[ct * P:(ct + 1) * P, j * XCW:(j + 1) * XCW])
    for w in range(WARMUP_MM):
        pw = ps_s.tile([P, P], F32, tag="s", name=f"warm{w}")
        nc.tensor.matmul(pw, lhsT=gmat_sb, rhs=gmat_sb, start=True, stop=True)
    for j in range(NB):
        for ct in range(NCT):
            csl = slice(j * NBLK, (j + 1) * NBLK)
            nc.vector.bn_stats(stats_t[ct][:, j, :], x_sb[ct][:, csl])
            nc.scalar.copy(x8[:, ct, csl], x_sb[ct][:, csl])

    # ---- GN stats -> per-channel scale/shift (h = x*sc + sh) ----
    scale_sh = []
    for ct in range(NCT):
        mv = small.tile([P, 2], F32, tag=f"mv{ct}")
        nc.vector.bn_aggr(mv, stats_t[ct])
        # t = [mean_c, E[x^2]_c]
        t = small.tile([P, 2], F32, tag=f"t{ct}")
        nc.vector.tensor_copy(t[:, 0:1], mv[:, 0:1])
        nc.vector.tensor_mul(t[:, 1:2], mv[:, 0:1], mv[:, 0:1])
        nc.vector.tensor_add(t[:, 1:2], t[:, 1:2], mv[:, 1:2])
        # group-average + broadcast back to channels via PE
        psg = ps_s.tile([P, 2], F32, tag="s")
        nc.tensor.matmul(psg, lhsT=gmat_sb, rhs=t, start=True, stop=True)
        g_sb = small.tile([P, 2], F32, tag=f"g{ct}")
        nc.vector.tensor_copy(g_sb, psg)
        # scale = gamma * rsqrt(var + eps);  shift = beta - group_mean * scale
        tmp = small.tile([P, 1], F32, tag=f"tmp{ct}")
        sc = small.tile([P, 1], F32, tag=f"sc{ct}")
        sh = small.tile([P, 1], F32, tag=f"sh{ct}")
        nc.vector.tensor_mul(tmp, g_sb[:, 0:1], g_sb[:, 0:1])
        nc.vector.tensor_tensor(tmp, g_sb[:, 1:2], tmp, ALU.subtract)  # var
        nc.scalar.activation(tmp, tmp, AF.Sqrt, bias=eps_t)
        nc.vector.reciprocal(tmp, tmp)                                 # rstd
        nc.vector.tensor_mul(sc, tmp, gamma_t[ct])
        nc.vector.tensor_mul(tmp, g_sb[:, 0:1], sc)
        nc.vector.tensor_tensor(sh, beta_t[ct], tmp, ALU.subtract)
        scale_sh.append((sc, sh))
    # switch the ACT table back to Exp now, off the critical path, so the
    # first softmax exp doesn't pay the 1.28us implicit load
    nc.scalar.activation(sqrt_dummy, eps_t, AF.Exp, scale=1.0)

    # ---- fold GN into the weights: wq8[c,:] = wqf[c,:] * sc[c] in fp8;
    # q bias' = W_q @ sh + b_q and bp_eff += W_vp @ sh via tiny matmuls ----
    wq8 = small.tile([P, 2, 3 * C], FP8, tag="wqkvT8", name="wq8")
    for ct in range(NCT):
        nc.vector.tensor_scalar(wq8[:, ct, :], wqf[:, ct, :], scale_sh[ct][0],
                                None, op0=ALU.mult)
    bias_q = []
    for o in range(NCT):
        psb = ps_s.tile([P, NBLK], F32, tag="s")
        for ct in range(NCT):
            nc.tensor.matmul(psb[:, 0:1], lhsT=wqf[:, ct, o * P:(o + 1) * P],
                             rhs=scale_sh[ct][1], start=(ct == 0),
                             stop=(ct == NCT - 1))
        bj = small.tile([P, 1], F32, tag=f"biasq{o}")
        nc.vector.tensor_add(bj, psb[:, 0:1], bq_in[o])
        bias_q.append(bj)
    bp_eff = []
    for o in range(NCT):
        psb = ps_s.tile([P, NBLK], F32, tag="s")
        for ct in range(NCT):
            nc.tensor.matmul(
                psb[:, 0:1],
                lhsT=wqf[:, ct, 2 * C + o * P:2 * C + (o + 1) * P],
                rhs=scale_sh[ct][1], start=(ct == 0), stop=(ct == NCT - 1))
        bj = small.tile([P, 1], F32, tag=f"bpe{o}")
        nc.vector.tensor_add(bj, psb[:, 0:1], bp_t[o])
        bp_eff.append(bj)

    # ---- qkv projections (GN pre-folded, so rhs is x8 directly). q/k land
    # in fp8 [128, 2, N] (channel-half on the middle dim) and vp in fp8
    # m-pair-interleaved [128, 2, 272] tiles so the attention matmuls can use
    # fp8 DoubleRow (2 values/PE-cell -> one matmul contracts 256). vp is the
    # w_proj-fused v projection; col 256 = ones (softmax denominators). ----
    q2 = big.tile([P, 2, N], FP8, tag="q2")
    k2 = big.tile([P, 2, N], FP8, tag="k2")
    VTW = 272
    vt_lo = big.tile([P, NM // 4, 2, VTW], FP8, tag="vt0", name="vt_lo")
    vt_hi = big.tile([P, NM // 4, 2, VTW], FP8, tag="vt1", name="vt_hi")
    # pre-fill the ones columns once (strided memsets, off-critical-path)
    nc.gpsimd.memset(vt_lo[:, :, :, C:C + 1], 1.0)
    nc.gpsimd.memset(vt_hi[:, :, :, C:C + 1], 1.0)

    def vt2(pair):
        return (vt_lo[:, pair] if pair < NM // 4
                else vt_hi[:, pair - NM // 4])

    def emit_q_blk(blk, only_o=None, eng=None):
        """q projection for block blk: 2 matmuls + 2 biased fp8 casts
        (ACT activation or DVE tensor_scalar; GPSIMD can't read PSUM).
        Deferrable (per channel-half) to just before block blk needs q2."""
        bsl = slice(blk * NBLK, (blk + 1) * NBLK)
        for o in range(NCT):
            if only_o is not None and o != only_o:
                continue
            ps = ps_s.tile([P, NBLK], F32, tag="s")
            nc.tensor.matmul(
                ps, lhsT=wq8[:, :, o * P:(o + 1) * P], rhs=x8[:, :, bsl],
                start=True, stop=True, perf_mode=DR)
            e = eng if eng is not None else (nc.scalar if o == 0 else
                                            nc.vector)
            if e is nc.scalar:
                nc.scalar.activation(q2[:, o, bsl], ps, AF.Identity,
                                     bias=bias_q[o], scale=1.0)
            else:
                e.tensor_scalar(q2[:, o, bsl], ps, bias_q[o], None,
                                op0=ALU.add)

    def emit_kv_blk(blk):
        """k + vp projections for block blk (biases cancel / fold away, so
        all copies are plain psum->fp8 casts, split 1/3 each across ACT,
        DVE and GPSIMD -- all six kv blocks beyond the first two must land
        inside attention block 0's window, alongside 32 exps)."""
        bsl = slice(blk * NBLK, (blk + 1) * NBLK)
        for o in range(NCT):
            ps = ps_s.tile([P, NBLK], F32, tag="s")
            nc.tensor.matmul(
                ps, lhsT=wq8[:, :, C + o * P:C + (o + 1) * P],
                rhs=x8[:, :, bsl], start=True, stop=True, perf_mode=DR)
            if o == 0:
                nc.scalar.copy(k2[:, o, bsl], ps)
            else:
                nc.vector.tensor_copy(k2[:, o, bsl], ps)
        for m in range(4 * blk, 4 * blk + 4):
            ps = ps_s.tile([P, NBLK], F32, tag="s")
            nc.tensor.matmul(
                ps[:, 0:C], lhsT=x8[:, :, m * P:(m + 1) * P],
                rhs=wq8[:, :, 2 * C:3 * C],
                start=True, stop=True, perf_mode=DR)
            dst = vt2(m // 2)[:, m % 2]
            if m % 2 == 0:
                nc.scalar.copy(dst[:, 0:C], ps[:, 0:C])
            else:
                nc.vector.tensor_copy(dst[:, 0:C], ps[:, 0:C])

    # ---- softmax divide + output helpers ----
    def emit_div_a(pend, csl=slice(0, NBLK)):
        pav0, pav1, psum, nb = pend
        w = csl.stop - csl.start
        sums_sb = work2.tile([1, NBLK], F32, tag="sums")
        nc.scalar.activation(sums_sb[:, csl], psum[:, csl], AF.Copy, bias=0.0)
        bc2 = work2.tile([P, NBLK], F32, tag="bc2")
        nc.gpsimd.partition_broadcast(bc2[:, csl], sums_sb[:, csl])
        bc_sb = work2.tile([P, NBLK], F32, tag="bc")
        nc.vector.reciprocal_approx_fast(bc_sb[:, csl], bc2[:, csl])
        return bc_sb

    def emit_div_b(pend, bc_sb, o, csl=slice(0, NBLK), dma_eng=None):
        """One output channel-half: divide + bias + residual + store.
        Split into two calls so the DVE burst spreads across the block."""
        pav = pend[o]
        nb = pend[3]
        nsl = slice(nb * NBLK + csl.start, nb * NBLK + csl.stop)
        t = tdiv.tile([P, NBLK], F32, tag="t")
        nc.vector.tensor_mul(t[:, csl], pav[:, csl], bc_sb[:, csl])
        st = stage.tile([P, NBLK], F32, tag="st")
        nc.vector.scalar_tensor_tensor(st[:, csl], t[:, csl], bp_eff[o],
                                       x_sb[o][:, nsl],
                                       op0=ALU.add, op1=ALU.add)
        if dma_eng is None:
            dma_eng = nc.sync if o == 0 else nc.gpsimd
        dma_eng.dma_start(out_d[o * P:(o + 1) * P, nsl], st[:, csl])

    # ---- global software-pipelined attention loop over gp = nb*16 + pair.
    # At iteration gp we emit: exps(gp+1) (engine queues run them while PE
    # works), the score matmuls for gp+2 (split around the avs so the bank
    # freed by exp0(gp+1) is reused late), and the av matmuls for gp (whose
    # e2 was exp'd during iteration gp-1 -> a full pair-period of exp slack,
    # so exp latency never stalls the PE). ----
    NPAIR = NM // 2          # 16 pairs per block
    NGP = NB * NPAIR         # 128
    ps_m = {}
    e2_pend = {}
    blk_tiles = {}

    def emit_scores(gp, half):
        if gp >= NGP:
            return
        nb, p = divmod(gp, NPAIR)
        m = 2 * p + half
        ps = ps_s.tile([P, NBLK], F32, tag="s")
        nc.tensor.matmul(ps, lhsT=k2[:, :, m * P:(m + 1) * P],
                         rhs=q2[:, :, nb * NBLK:(nb + 1) * NBLK],
                         start=True, stop=True, perf_mode=DR)
        ps_m[(gp, half)] = ps

    def dve_pair(gp):
        # Runs of 2 pairs per engine: short enough that the queue lag stays
        # within the 3-bank slack, long enough that ACT and DVE are not hot
        # simultaneously (concurrent ACT+DVE activity drops the chip clock
        # from 2.4 to 2.0 GHz -- measured 379ns vs 454ns matmuls). 8/8
        # split everywhere: block 0 used to give DVE only 4 pairs, which
        # left ACT with 24 exps + kv copies (~26us of work in a 23us
        # window) and stalled the av matmuls behind the exps.
        nb, p = divmod(gp, NPAIR)
        return p % 4 >= 2

    def emit_exps(gp):
        if gp >= NGP:
            return
        on_dve = dve_pair(gp)
        e2 = work.tile([P, 2, NBLK], FP8, tag="e")
        for half in range(2):
            ps = ps_m.pop((gp, half))
            if on_dve:
                nc.vector._custom_dve(EXP8, out=e2[:, half], in0=ps,
                                      s0=EXP_C0, s1=_EXP_C1, imm2=_EXP_C2)
            else:
                nc.scalar.activation(e2[:, half], ps, AF.Exp,
                                     scale=float(SCALE))
        e2_pend[gp] = e2

    def emit_avs(gp):
        nb, p = divmod(gp, NPAIR)
        e2 = e2_pend.pop(gp)
        pav0, pav1, psum = blk_tiles[nb]
        first, last = (p == 0), (p == NPAIR - 1)
        vtp = vt2(p)
        nc.tensor.matmul(pav0, lhsT=vtp[:, :, 0:P], rhs=e2,
                         start=first, stop=last, perf_mode=DR)
        nc.tensor.matmul(pav1, lhsT=vtp[:, :, P:2 * P], rhs=e2,
                         start=first, stop=last, perf_mode=DR)
        nc.tensor.matmul(psum, lhsT=vtp[:, :, 2 * P:2 * P + 1], rhs=e2,
                         start=first, stop=last, perf_mode=DR)

    def new_blk_tiles(nb):
        pav0 = ps_av0.tile([P, NBLK], F32, tag="av0", name=f"av0_{nb}")
        pav1 = ps_av1.tile([P, NBLK], F32, tag="av1", name=f"av1_{nb}")
        psum = ps_sum.tile([1, NBLK], F32, tag="sum", name=f"sum_{nb}")
        blk_tiles[nb] = (pav0, pav1, psum)

    # Fused phase gating: kv block b unlocks k2 m-tiles < 4(b+1) and vt
    # pairs < 2(b+1); at iteration gp of block 0 the scores reach m-tile
    # 2*gp+5 and the avs read vt pair gp, both covered once kv blocks
    # <= gp/2 + 2 are in.
    emit_q_blk(0)
    emit_kv_blk(0)
    emit_kv_blk(1)
    emit_scores(0, 0)
    emit_scores(0, 1)
    emit_exps(0)
    emit_scores(1, 0)
    emit_scores(1, 1)
    emit_q_blk(1)

    state = {"pend": None, "bc_prev": None}
    for gp in range(NGP):
        nb, p = divmod(gp, NPAIR)
        if nb not in blk_tiles:
            new_blk_tiles(nb)
        if nb == 0:
            if p % 2 == 0 and 2 + p // 2 < NB:
                emit_kv_blk(2 + p // 2)
            if p == 12:
                emit_q_blk(2)
        emit_exps(gp + 1)
        emit_scores(gp + 2, 0)
        if p == 4 and nb > 0:
            emit_div_b(state["pend"], state["bc_prev"], 0)
        if p == 8 and nb > 0:
            emit_div_b(state["pend"], state["bc_prev"], 1)
        if p == 2 and nb >= 2 and nb + 1 < NB:
            emit_q_blk(nb + 1, only_o=0)
        if p == 6 and nb >= 2 and nb + 1 < NB:
            emit_q_blk(nb + 1, only_o=1)
        emit_avs(gp)
        emit_scores(gp + 2, 1)
        if p == NPAIR - 1 and nb < NB - 1:
            # div_a emitted before the next block's first den matmul can
            # touch the single-buffered ps_sum bank
            state["pend"] = (*blk_tiles.pop(nb), nb)
            state["bc_prev"] = emit_div_a(state["pend"])
    # ---- tail: the last block's divide runs with nothing behind it, so
    # chunk it in half-width pieces pipelined across ACT/GPS/DVE and spread
    # the final stores over four DMA queues. ----
    pend = (*blk_tiles.pop(NB - 1), NB - 1)
    HB = NBLK // 2
    csls = [slice(0, HB), slice(HB, NBLK)]
    bcs = [emit_div_a(pend, csl) for csl in csls]
    dqs = [nc.sync, nc.gpsimd, nc.scalar, nc.sync]
    for o in range(NCT):
        for ci, csl in enumerate(csls):
            emit_div_b(pend, bcs[ci], o, csl, dma_eng=dqs[2 * o + ci])


def build_nc() -> bass.Bass:
    nc = bacc.Bacc("TRN2", target_bir_lowering=False, debug=False)
    x = nc.dram_tensor("x", [C, N], F32, kind="ExternalInput")
    wqkvT = nc.dram_tensor("wqkvT", [P, 2, 3 * C], F32, kind="ExternalInput")
    consts = nc.dram_tensor("consts", [P, 8], F32, kind="ExternalInput")
    gmat = nc.dram_tensor("gmat", [P, P], F32, kind="ExternalInput")
    out = nc.dram_tensor("out", [C, N], F32, kind="ExternalOutput")
    with tile.TileContext(nc) as tc:
        emit_kernel(tc, out.ap(), x.ap(), wqkvT.ap(), consts.ap(), gmat.ap())
    nc.compile()
    return nc


_NC_CACHE: list = []


def _in_maps(x, gamma, beta, w_qkv, b_qkv, w_proj, b_proj):
    f = lambda a: np.ascontiguousarray(np.asarray(a, dtype=np.float32))
    xs = f(x).reshape(B, C, N)
    w_qkv = np.asarray(w_qkv, dtype=np.float64)
    w_proj = np.asarray(w_proj, dtype=np.float64)
    b_qkv = np.asarray(b_qkv, dtype=np.float64)
    b_proj = np.asarray(b_proj, dtype=np.float64)
    # fuse w_proj into the v projection; its bias rides into bproj (softmax
    # rows sum to 1, so a constant vp offset is a constant output offset)
    w_fused = np.concatenate(
        [w_qkv[0:2 * C], w_proj @ w_qkv[2 * C:3 * C]], axis=0)
    bp_eff = b_proj + w_proj @ b_qkv[2 * C:3 * C]
    # consts cols: gamma0 gamma1 beta0 beta1 bp0 bp1 bq0 bq1
    gamma = np.asarray(gamma, np.float64)
    beta = np.asarray(beta, np.float64)
    consts = np.stack(
        [gamma[0:P], gamma[P:C], beta[0:P], beta[P:C],
         bp_eff[0:P], bp_eff[P:C], b_qkv[0:P], b_qkv[P:C]], axis=1)
    base = {
        "wqkvT": f(w_fused.T.reshape(2, P, 3 * C).transpose(1, 0, 2)),
        "consts": f(consts),
        "gmat": _group_mat(),
    }
    return [{**base, "x": np.ascontiguousarray(xs[i])} for i in range(B)]


def run_spmd(x, gamma, beta, w_qkv, b_qkv, w_proj, b_proj, **kwargs):
    from concourse.bass_utils import run_bass_kernel_spmd

    if not _NC_CACHE:
        _NC_CACHE.append(build_nc())
    nc = _NC_CACHE[0]
    maps = _in_maps(x, gamma, beta, w_qkv, b_qkv, w_proj, b_proj)
    res = run_bass_kernel_spmd(nc, maps, core_ids=list(range(B)), **kwargs)
    out = np.stack([res.results[i]["out"] for i in range(B)])
    return out.reshape(B, C, H, W), res


def kernel(x, gamma, beta, w_qkv, b_qkv, w_proj, b_proj) -> np.ndarray:
    out, _ = run_spmd(x, gamma, beta, w_qkv, b_qkv, w_proj, b_proj)
    return out

